# revision 2
# baseline (speedup 1.0000x reference)
"""Grid-accelerated KDTree-distance-loss kernel for Trainium2 (8 cores, SPMD).

Math: for each src point (16384 x 3), min over tgt (16384 x 3) of ||s-t||^2,
clamp (>1.0 -> 0), mean.

Algorithm (exact, clamp-aware):
  Host builds a uniform grid over tgt at cell sides h in {0.25, 0.5, 1.0}.
  For a src point in cell c (side h), every tgt outside the 3x3x3
  neighborhood N(c) is at distance >= h. A cheap host-side witness (distance
  to <=32 nearest-cell candidates) proves min <= h^2 for most src at
  h=0.25; the rest escalate. At the terminal level h=1.0, either
  min(candidates) <= 1.0 (exact) or the true min > 1.0 and the clamp sends
  it to 0 -- so three levels give the exact clamped loss.

  Src are Morton-sorted and packed into blocks of <=128 sharing the union
  of their candidate lists (bbox-ball filtered by the witness radius).
  Blocks are chunked to <=1024 columns and bin-packed into fixed
  128-row x 1024-col slots; extra fp16 "mask rows" in the augmented
  matmul add +49152 at foreign columns so several blocks can share one
  slot. The device computes, per slot, q[p, j] = -2 s_p . t_j + |t_j|^2
  via an 11+6-row hi/lo-split fp16 matmul (PSUM fp32, ~1e-5 abs accuracy)
  and reduces min_j q with one DVE tensor_tensor_scan over the two
  512-halves of the PSUM tile (one read direct from PSUM, one staged to
  SBUF by the scalar engine). Host combines slot minima per src, adds
  |s|^2, clamps, means.
"""

import numpy as np

import concourse.bacc as bacc
import concourse.mybir as mybir
from concourse.tile import TileContext

N_CORES = 8
P = 128                # src points per block (partition dim)
C = 1024               # candidate columns per slot (2 segments x 512)
HALF = C // 2
G_MAX = 6              # max sub-blocks (pieces) per slot
K_AUG = 11             # hilo fp16 augmented rows
K = K_AUG + G_MAX      # + per-piece mask rows
LEVELS = (0.25, 0.5, 1.0)
WITNESS = 32
ORIGIN = -8.0
PAD_Q = 60000.0        # q value of the padding candidate column (fp16-safe)
MASK_BIG = 49152.0     # exact in fp16; added to q at foreign columns

_CACHE = {}


# ----------------------------------------------------------------- device ---

def build(nslots):
    """Bass module: nslots independent (<=128 src x 1024 cand) min-reductions.
    Per slot: two 512-col matmuls into one PSUM tile, Act stages the odd
    half to SBUF, one 512-wide DVE min-scan folds both halves."""
    f16 = mybir.dt.float16
    f32 = mybir.dt.float32
    MIN = mybir.AluOpType.min

    nc = bacc.Bacc(None)
    lhs_d = nc.declare_dram_parameter("lhs", [K, nslots * P], f16, isOutput=False)
    rhs_d = nc.declare_dram_parameter("rhs", [K, nslots * C], f16, isOutput=False)
    out_d = nc.declare_dram_parameter("out", [P, nslots], f32, isOutput=True)

    with TileContext(nc) as tc:
        with (
            tc.tile_pool(name="const", bufs=1) as const_pool,
            tc.tile_pool(name="psum", bufs=4, space="PSUM") as psum_pool,
            tc.tile_pool(name="stage", bufs=4) as stage_pool,
        ):
            lhs = const_pool.tile([K, nslots * P], f16, tag="lhs")
            nc.sync.dma_start(lhs[:, :], lhs_d[:, :])
            rhs = const_pool.tile([K, nslots * C], f16, tag="rhs")
            # slot-aligned rhs chunks, small first so compute starts early
            chunks = []
            s0 = 0
            for sz in (1, 2, 4):
                if s0 + sz <= nslots:
                    chunks.append((s0, s0 + sz))
                    s0 += sz
            while s0 < nslots:
                sz = min(8, nslots - s0)
                chunks.append((s0, s0 + sz))
                s0 += sz
            for lo, hi in chunks:
                nc.sync.dma_start(rhs[:, lo * C:hi * C], rhs_d[:, lo * C:hi * C])
            so_all = const_pool.tile([P, nslots, HALF], f32, tag="so")

            half_s = nslots // 2
            for s in range(nslots):
                w = lhs[:, s * P:(s + 1) * P]
                p = psum_pool.tile([P, C], f32)
                nc.tensor.matmul(p[:, 0:HALF], w, rhs[:, s * C:s * C + HALF],
                                 start=True, stop=True)
                nc.tensor.matmul(p[:, HALF:C], w, rhs[:, s * C + HALF:(s + 1) * C],
                                 start=True, stop=True)
                c = stage_pool.tile([P, HALF], f32)
                nc.scalar.copy(c[:, :], p[:, HALF:C])
                nc.vector.tensor_tensor_scan(
                    out=so_all[:, s, :], data0=p[:, 0:HALF], data1=c[:, :],
                    initial=3.0e38, op0=MIN, op1=MIN,
                )
                if s == half_s - 1 and half_s > 0:
                    # first half of results leaves while the rest computes
                    nc.sync.dma_start(out_d[:, 0:half_s],
                                      so_all[:, 0:half_s, HALF - 1:HALF])
            nc.sync.dma_start(out_d[:, half_s:nslots],
                              so_all[:, half_s:nslots, HALF - 1:HALF])
    nc.compile()
    return nc


def _get_nc(nslots):
    key = ("nc", nslots)
    if key not in _CACHE:
        _CACHE[key] = build(nslots)
    return _CACHE[key]


# ------------------------------------------------------------ host indexing ---

def _morton(ci):
    def spread(x):
        x = x.astype(np.uint64)
        x = (x | (x << np.uint64(16))) & np.uint64(0x30000FF)
        x = (x | (x << np.uint64(8))) & np.uint64(0x300F00F)
        x = (x | (x << np.uint64(4))) & np.uint64(0x30C30C3)
        x = (x | (x << np.uint64(2))) & np.uint64(0x9249249)
        return x
    return (spread(ci[:, 0]) | (spread(ci[:, 1]) << np.uint64(1))
            | (spread(ci[:, 2]) << np.uint64(2)))


def _build_level(src_pts, tgt, h):
    nside = int(np.ceil(16.0 / h))
    ci_s = np.floor((np.clip(src_pts, -7.99, 7.99) - ORIGIN) / h).astype(np.int64)
    ci_t = np.floor((np.clip(tgt, -7.99, 7.99) - ORIGIN) / h).astype(np.int64)
    key_s = (ci_s[:, 0] * nside + ci_s[:, 1]) * nside + ci_s[:, 2]
    key_t = (ci_t[:, 0] * nside + ci_t[:, 1]) * nside + ci_t[:, 2]
    t_order = np.argsort(key_t, kind="stable")
    kt_sorted = key_t[t_order]
    trip = [(a, b, c) for a in (-1, 0, 1) for b in (-1, 0, 1) for c in (-1, 0, 1)]
    trip.sort(key=lambda t: abs(t[0]) + abs(t[1]) + abs(t[2]))
    offs = np.array([(a * nside + b) * nside + c for a, b, c in trip])
    return key_s, kt_sorted, t_order, offs, ci_s


def _cands_of_cell(u, kt_sorted, t_order, offs):
    segs = []
    for o in offs:
        lo = np.searchsorted(kt_sorted, u + o, side="left")
        hi = np.searchsorted(kt_sorted, u + o, side="right")
        if hi > lo:
            segs.append(t_order[lo:hi])
    return np.concatenate(segs) if segs else np.empty(0, np.int64)


def build_slots(src, tgt):
    """Returns slots: list of slot = list of pieces (src_idx<=P, cand_idx<=C).
    Pieces in one slot are from different blocks; mask rows keep them apart.

    Exactness: for each src s, its piece's candidate set contains every tgt
    within min(witness_dist(s), 1.0) of s (bbox-ball filter with radius
    r_blk = max over block members). So the computed min is the true min
    whenever the true min <= 1.0; otherwise both are > 1.0 -> clamp 0.
    """
    src64 = src.astype(np.float64)
    tgt64 = tgt.astype(np.float64)
    n = len(src64)
    remaining = np.arange(n)
    wit_d2 = np.full(n, np.inf)
    blocks = []  # (src_idx, cand_idx filtered)
    for li, h in enumerate(LEVELS):
        terminal = li == len(LEVELS) - 1
        if len(remaining) == 0:
            break
        key_s, kt_sorted, t_order, offs, ci_s = _build_level(src64[remaining], tgt64, h)
        uniq, inv = np.unique(key_s, return_inverse=True)
        cands = {u: _cands_of_cell(u, kt_sorted, t_order, offs) for u in uniq}
        guaranteed = np.zeros(len(remaining), bool)
        for i, u in enumerate(uniq):
            rows = np.where(inv == i)[0]
            cl = cands[u][:WITNESS]
            if len(cl) == 0:
                guaranteed[rows] = terminal
                continue
            d2 = ((src64[remaining[rows], None, :] - tgt64[None, cl, :]) ** 2
                  ).sum(-1).min(1)
            wit_d2[remaining[rows]] = np.minimum(wit_d2[remaining[rows]], d2)
            guaranteed[rows] = terminal or (d2 <= h * h)
        g_rows = np.where(guaranteed)[0]
        if len(g_rows):
            mort = _morton(ci_s[g_rows])
            g_sorted = g_rows[np.argsort(mort, kind="stable")]
            for b0 in range(0, len(g_sorted), P):
                mem = g_sorted[b0:b0 + P]
                sidx = remaining[mem]
                cl = np.unique(np.concatenate([cands[key_s[m]] for m in mem]))
                if len(cl):
                    # bbox-ball filter: keep t with d(t, bbox)^2 <= r^2
                    pts = src64[sidx]
                    lo, hi = pts.min(0), pts.max(0)
                    r2 = np.minimum(wit_d2[sidx], 1.0).max() + 1e-9
                    tc = tgt64[cl]
                    dv = np.maximum(np.maximum(lo[None, :] - tc, tc - hi[None, :]), 0.0)
                    cl = cl[(dv ** 2).sum(1) <= r2]
                blocks.append((sidx, cl))
        remaining = remaining[~guaranteed]
    assert len(remaining) == 0

    # pieces: chunk block candidate lists to <= C columns
    pieces = []
    for bi, (src_idx, cl) in enumerate(blocks):
        if len(cl) == 0:
            cl = np.array([len(tgt)], np.int64)  # pad column only
        for c0 in range(0, len(cl), C):
            pieces.append((bi, src_idx, cl[c0:c0 + C]))
    # first-fit-decreasing bin packing: rows<=P, cols<=C, pieces<=G_MAX,
    # no two pieces of one block in the same slot
    pieces.sort(key=lambda t: -len(t[2]))
    slots = []  # each: [rows, cols, set(block_ids), [(src_idx, cand_idx)]]
    for bi, sidx, cl in pieces:
        for sl in slots:
            if (sl[0] + len(sidx) <= P and sl[1] + len(cl) <= C
                    and bi not in sl[2] and len(sl[3]) < G_MAX):
                sl[0] += len(sidx)
                sl[1] += len(cl)
                sl[2].add(bi)
                sl[3].append((sidx, cl))
                break
        else:
            slots.append([len(sidx), len(cl), {bi}, [(sidx, cl)]])
    return [sl[3] for sl in slots]


# ------------------------------------------------------------------- glue ---

def _prep_aug(src, tgt):
    """hilo fp16 augmentation. lhsT [K_AUG, N], rhs [K_AUG, M+1] (pad last)."""
    src = np.asarray(src, np.float32)
    tgt = np.asarray(tgt, np.float32)
    n, m = src.shape[0], tgt.shape[0]
    u = (-2.0 * tgt.astype(np.float64)).astype(np.float32)
    t2 = (tgt.astype(np.float64) ** 2).sum(1).astype(np.float32)
    hs = src.astype(np.float16)
    ls = (src - hs.astype(np.float32)).astype(np.float16)
    hu = u.astype(np.float16)
    lu = (u - hu.astype(np.float32)).astype(np.float16)
    t2h = t2.astype(np.float16)
    t2l = (t2 - t2h.astype(np.float32)).astype(np.float16)
    lhsT = np.empty((K_AUG, n), np.float16)
    lhsT[0:3] = hs.T
    lhsT[3:6] = ls.T
    lhsT[6:9] = hs.T
    lhsT[9] = np.float16(1.0)
    lhsT[10] = np.float16(1.0)
    rhs = np.zeros((K_AUG, m + 1), np.float16)
    rhs[0:3, :m] = hu.T
    rhs[3:6, :m] = hu.T
    rhs[6:9, :m] = lu.T
    rhs[9, :m] = t2h
    rhs[10, :m] = t2l
    rhs[9, m] = np.float16(PAD_Q)
    return lhsT, rhs


def _run_device(src, tgt, trace=False):
    from concourse.bass_utils import run_bass_kernel_spmd

    src = np.asarray(src, np.float32)
    tgt = np.asarray(tgt, np.float32)
    n, m = src.shape[0], tgt.shape[0]
    lhsT, rhs = _prep_aug(src, tgt)
    slots = build_slots(src, tgt)
    nslots = int(np.ceil(len(slots) / N_CORES))

    in_maps = []
    slot_of_core = []
    for core in range(N_CORES):
        lhs_a = np.zeros((K, nslots * P), np.float16)
        rhs_a = np.zeros((K, nslots * C), np.float16)
        rhs_a[:K_AUG] = rhs[:, m:m + 1]  # default: pad column
        csl = slots[core * nslots:(core + 1) * nslots]
        placed = []
        for si, pieces in enumerate(csl):
            ro = co = 0
            ranges = []
            pl = []
            for src_idx, cand_idx in pieces:
                lr, lc = len(src_idx), len(cand_idx)
                lhs_a[:K_AUG, si * P + ro:si * P + ro + lr] = lhsT[:, src_idx]
                rhs_a[:K_AUG, si * C + co:si * C + co + lc] = rhs[:, cand_idx]
                pl.append((src_idx, ro))
                ranges.append((ro, lr, co, lc))
                ro += lr
                co += lc
            used = co
            for g, (gro, glr, gco, glc) in enumerate(ranges):
                lhs_a[K_AUG + g, si * P + gro:si * P + gro + glr] = MASK_BIG
                rhs_a[K_AUG + g, si * C:si * C + used] = 1.0
                rhs_a[K_AUG + g, si * C + gco:si * C + gco + glc] = 0.0
            placed.append(pl)
        slot_of_core.append(placed)
        in_maps.append({"lhs": lhs_a, "rhs": rhs_a})

    nc = _get_nc(nslots)
    r = run_bass_kernel_spmd(nc, in_maps, list(range(N_CORES)), trace=trace)

    minq = np.full(n, np.inf, np.float32)
    for core in range(N_CORES):
        out = np.asarray(r.results[core]["out"])  # [P, nslots]
        for si, pl in enumerate(slot_of_core[core]):
            for src_idx, ro in pl:
                np.minimum.at(minq, src_idx, out[ro:ro + len(src_idx), si])
    return minq, r, nc


def _finish(minq, src):
    src = np.asarray(src, np.float32)
    s2 = (src.astype(np.float64) ** 2).sum(1).astype(np.float32)
    d2 = np.maximum(minq + s2, 0.0)
    clamped = np.where(d2 > 1.0, 0.0, d2)
    return np.float32(clamped.mean(dtype=np.float64))


def kernel(src, tgt, idx=None, **_ignored):
    minq, _, _ = _run_device(src, tgt)
    return np.asarray(_finish(minq, src))


def kernel_traced(src, tgt, idx=None):
    minq, r, nc = _run_device(src, tgt, trace=True)
    return np.asarray(_finish(minq, src)), r, nc


# revision 3
# speedup vs baseline: 1.1249x; 1.1249x over previous
"""Grid-accelerated KDTree-distance-loss kernel for Trainium2 (8 cores, SPMD).

Math: for each src point (16384 x 3), min over tgt (16384 x 3) of ||s-t||^2,
clamp (>1.0 -> 0), mean.

Algorithm (exact, clamp-aware):
  Host builds a uniform grid over tgt at cell sides h in {0.25, 0.5, 1.0}.
  For a src point in cell c (side h), every tgt outside the 3x3x3
  neighborhood N(c) is at distance >= h. A cheap host-side witness (distance
  to <=32 nearest-cell candidates) proves min <= h^2 for most src at
  h=0.25; the rest escalate. At the terminal level h=1.0, either
  min(candidates) <= 1.0 (exact) or the true min > 1.0 and the clamp sends
  it to 0 -- so three levels give the exact clamped loss.

  Src are Morton-sorted and packed into blocks of <=128 sharing the union
  of their candidate lists (bbox-ball filtered by the witness radius).
  Blocks are chunked to <=1024 columns and bin-packed into fixed
  128-row x 1024-col slots; extra fp16 "mask rows" in the augmented
  matmul add +49152 at foreign columns so several blocks can share one
  slot. The device computes, per slot, q[p, j] = -2 s_p . t_j + |t_j|^2
  via an 11+6-row hi/lo-split fp16 matmul (PSUM fp32, ~1e-5 abs accuracy)
  and reduces min_j q with one DVE tensor_tensor_scan over the two
  512-halves of the PSUM tile (one read direct from PSUM, one staged to
  SBUF by the scalar engine). Host combines slot minima per src, adds
  |s|^2, clamps, means.
"""

import numpy as np

import concourse.bacc as bacc
import concourse.mybir as mybir
from concourse.tile import TileContext

N_CORES = 8
P = 128                # src points per block (partition dim)
C = 1024               # candidate columns per slot (2 segments x 512)
HALF = C // 2
G_MAX = 6              # max sub-blocks (pieces) per slot
K_AUG = 11             # hilo fp16 augmented rows
K = K_AUG + G_MAX      # + per-piece mask rows
LEVELS = (0.25, 0.5, 1.0)
WITNESS = 32
ORIGIN = -8.0
PAD_Q = 60000.0        # q value of the padding candidate column (fp16-safe)
MASK_BIG = 49152.0     # exact in fp16; added to q at foreign columns

_CACHE = {}


# ----------------------------------------------------------------- device ---

def build(nslots):
    """Bass module: nslots independent (<=128 src x 1024 cand) min-reductions.
    Per slot: two 512-col matmuls into one PSUM tile, Act stages the odd
    half to SBUF, one 512-wide DVE min-scan folds both halves."""
    f16 = mybir.dt.float16
    f32 = mybir.dt.float32
    MIN = mybir.AluOpType.min

    nc = bacc.Bacc(None)
    lhs_d = nc.declare_dram_parameter("lhs", [K, nslots * P], f16, isOutput=False)
    rhs_d = nc.declare_dram_parameter("rhs", [K, nslots * C], f16, isOutput=False)
    out_d = nc.declare_dram_parameter("out", [P, nslots], f32, isOutput=True)

    with TileContext(nc) as tc:
        with (
            tc.tile_pool(name="const", bufs=1) as const_pool,
            tc.tile_pool(name="psum", bufs=4, space="PSUM") as psum_pool,
            tc.tile_pool(name="stage", bufs=4) as stage_pool,
        ):
            lhs = const_pool.tile([K, nslots * P], f16, tag="lhs")
            nc.sync.dma_start(lhs[:, :], lhs_d[:, :])
            rhs = const_pool.tile([K, nslots * C], f16, tag="rhs")
            # slot-aligned rhs chunks, small first so compute starts early
            chunks = []
            s0 = 0
            for sz in (1, 2, 4):
                if s0 + sz <= nslots:
                    chunks.append((s0, s0 + sz))
                    s0 += sz
            while s0 < nslots:
                sz = min(8, nslots - s0)
                chunks.append((s0, s0 + sz))
                s0 += sz
            for lo, hi in chunks:
                nc.sync.dma_start(rhs[:, lo * C:hi * C], rhs_d[:, lo * C:hi * C])
            so_all = const_pool.tile([P, nslots, HALF], f32, tag="so")

            half_s = nslots // 2
            for s in range(nslots):
                w = lhs[:, s * P:(s + 1) * P]
                p = psum_pool.tile([P, C], f32)
                nc.tensor.matmul(p[:, 0:HALF], w, rhs[:, s * C:s * C + HALF],
                                 start=True, stop=True)
                nc.tensor.matmul(p[:, HALF:C], w, rhs[:, s * C + HALF:(s + 1) * C],
                                 start=True, stop=True)
                c = stage_pool.tile([P, HALF], f32)
                nc.scalar.copy(c[:, :], p[:, HALF:C])
                nc.vector.tensor_tensor_scan(
                    out=so_all[:, s, :], data0=p[:, 0:HALF], data1=c[:, :],
                    initial=3.0e38, op0=MIN, op1=MIN,
                )
                if s == half_s - 1 and half_s > 0:
                    # first half of results leaves while the rest computes
                    nc.sync.dma_start(out_d[:, 0:half_s],
                                      so_all[:, 0:half_s, HALF - 1:HALF])
            if nslots - 1 > half_s:
                nc.sync.dma_start(out_d[:, half_s:nslots - 1],
                                  so_all[:, half_s:nslots - 1, HALF - 1:HALF])
            # tail DMA covers only the last slot: minimal post-scan chain
            nc.sync.dma_start(out_d[:, nslots - 1:nslots],
                              so_all[:, nslots - 1:nslots, HALF - 1:HALF])
    nc.compile()
    return nc


def _get_nc(nslots):
    key = ("nc", nslots)
    if key not in _CACHE:
        _CACHE[key] = build(nslots)
    return _CACHE[key]


# ------------------------------------------------------------ host indexing ---

def _morton(ci):
    def spread(x):
        x = x.astype(np.uint64)
        x = (x | (x << np.uint64(16))) & np.uint64(0x30000FF)
        x = (x | (x << np.uint64(8))) & np.uint64(0x300F00F)
        x = (x | (x << np.uint64(4))) & np.uint64(0x30C30C3)
        x = (x | (x << np.uint64(2))) & np.uint64(0x9249249)
        return x
    return (spread(ci[:, 0]) | (spread(ci[:, 1]) << np.uint64(1))
            | (spread(ci[:, 2]) << np.uint64(2)))


def _build_level(src_pts, tgt, h):
    nside = int(np.ceil(16.0 / h))
    ci_s = np.floor((np.clip(src_pts, -7.99, 7.99) - ORIGIN) / h).astype(np.int64)
    ci_t = np.floor((np.clip(tgt, -7.99, 7.99) - ORIGIN) / h).astype(np.int64)
    key_s = (ci_s[:, 0] * nside + ci_s[:, 1]) * nside + ci_s[:, 2]
    key_t = (ci_t[:, 0] * nside + ci_t[:, 1]) * nside + ci_t[:, 2]
    t_order = np.argsort(key_t, kind="stable")
    kt_sorted = key_t[t_order]
    trip = [(a, b, c) for a in (-1, 0, 1) for b in (-1, 0, 1) for c in (-1, 0, 1)]
    trip.sort(key=lambda t: abs(t[0]) + abs(t[1]) + abs(t[2]))
    offs = np.array([(a * nside + b) * nside + c for a, b, c in trip])
    return key_s, kt_sorted, t_order, offs, ci_s


def _cands_of_cell(u, kt_sorted, t_order, offs):
    segs = []
    for o in offs:
        lo = np.searchsorted(kt_sorted, u + o, side="left")
        hi = np.searchsorted(kt_sorted, u + o, side="right")
        if hi > lo:
            segs.append(t_order[lo:hi])
    return np.concatenate(segs) if segs else np.empty(0, np.int64)


def build_slots(src, tgt):
    """Returns slots: list of slot = list of pieces (src_idx<=P, cand_idx<=C).
    Pieces in one slot are from different blocks; mask rows keep them apart.

    Exactness: for each src s, its piece's candidate set contains every tgt
    within min(witness_dist(s), 1.0) of s (bbox-ball filter with radius
    r_blk = max over block members). So the computed min is the true min
    whenever the true min <= 1.0; otherwise both are > 1.0 -> clamp 0.
    """
    src64 = src.astype(np.float64)
    tgt64 = tgt.astype(np.float64)
    n = len(src64)
    remaining = np.arange(n)
    wit_d2 = np.full(n, np.inf)
    blocks = []  # (src_idx, cand_idx filtered)
    for li, h in enumerate(LEVELS):
        terminal = li == len(LEVELS) - 1
        if len(remaining) == 0:
            break
        key_s, kt_sorted, t_order, offs, ci_s = _build_level(src64[remaining], tgt64, h)
        uniq, inv = np.unique(key_s, return_inverse=True)
        cands = {u: _cands_of_cell(u, kt_sorted, t_order, offs) for u in uniq}
        guaranteed = np.zeros(len(remaining), bool)
        for i, u in enumerate(uniq):
            rows = np.where(inv == i)[0]
            cl = cands[u][:WITNESS]
            if len(cl) == 0:
                guaranteed[rows] = terminal
                continue
            d2 = ((src64[remaining[rows], None, :] - tgt64[None, cl, :]) ** 2
                  ).sum(-1).min(1)
            wit_d2[remaining[rows]] = np.minimum(wit_d2[remaining[rows]], d2)
            guaranteed[rows] = terminal or (d2 <= h * h)
        g_rows = np.where(guaranteed)[0]
        if len(g_rows):
            mort = _morton(ci_s[g_rows])
            g_sorted = g_rows[np.argsort(mort, kind="stable")]
            for b0 in range(0, len(g_sorted), P):
                mem = g_sorted[b0:b0 + P]
                sidx = remaining[mem]
                cl = np.unique(np.concatenate([cands[key_s[m]] for m in mem]))
                if len(cl):
                    # bbox-ball filter: keep t with d(t, bbox)^2 <= r^2
                    pts = src64[sidx]
                    lo, hi = pts.min(0), pts.max(0)
                    r2 = np.minimum(wit_d2[sidx], 1.0).max() + 1e-9
                    tc = tgt64[cl]
                    dv = np.maximum(np.maximum(lo[None, :] - tc, tc - hi[None, :]), 0.0)
                    cl = cl[(dv ** 2).sum(1) <= r2]
                blocks.append((sidx, cl))
        remaining = remaining[~guaranteed]
    assert len(remaining) == 0

    # pieces: chunk block candidate lists to <= C columns
    pieces = []
    for bi, (src_idx, cl) in enumerate(blocks):
        if len(cl) == 0:
            cl = np.array([len(tgt)], np.int64)  # pad column only
        for c0 in range(0, len(cl), C):
            pieces.append((bi, src_idx, cl[c0:c0 + C]))
    # first-fit-decreasing bin packing: rows<=P, cols<=C, pieces<=G_MAX,
    # no two pieces of one block in the same slot
    pieces.sort(key=lambda t: -len(t[2]))
    slots = []  # each: [rows, cols, set(block_ids), [(src_idx, cand_idx)]]
    for bi, sidx, cl in pieces:
        for sl in slots:
            if (sl[0] + len(sidx) <= P and sl[1] + len(cl) <= C
                    and bi not in sl[2] and len(sl[3]) < G_MAX):
                sl[0] += len(sidx)
                sl[1] += len(cl)
                sl[2].add(bi)
                sl[3].append((sidx, cl))
                break
        else:
            slots.append([len(sidx), len(cl), {bi}, [(sidx, cl)]])
    return [sl[3] for sl in slots]


# ------------------------------------------------------------------- glue ---

def _prep_aug(src, tgt):
    """hilo fp16 augmentation. lhsT [K_AUG, N], rhs [K_AUG, M+1] (pad last)."""
    src = np.asarray(src, np.float32)
    tgt = np.asarray(tgt, np.float32)
    n, m = src.shape[0], tgt.shape[0]
    u = (-2.0 * tgt.astype(np.float64)).astype(np.float32)
    t2 = (tgt.astype(np.float64) ** 2).sum(1).astype(np.float32)
    hs = src.astype(np.float16)
    ls = (src - hs.astype(np.float32)).astype(np.float16)
    hu = u.astype(np.float16)
    lu = (u - hu.astype(np.float32)).astype(np.float16)
    t2h = t2.astype(np.float16)
    t2l = (t2 - t2h.astype(np.float32)).astype(np.float16)
    lhsT = np.empty((K_AUG, n), np.float16)
    lhsT[0:3] = hs.T
    lhsT[3:6] = ls.T
    lhsT[6:9] = hs.T
    lhsT[9] = np.float16(1.0)
    lhsT[10] = np.float16(1.0)
    rhs = np.zeros((K_AUG, m + 1), np.float16)
    rhs[0:3, :m] = hu.T
    rhs[3:6, :m] = hu.T
    rhs[6:9, :m] = lu.T
    rhs[9, :m] = t2h
    rhs[10, :m] = t2l
    rhs[9, m] = np.float16(PAD_Q)
    return lhsT, rhs


def _run_device(src, tgt, trace=False):
    from concourse.bass_utils import run_bass_kernel_spmd

    src = np.asarray(src, np.float32)
    tgt = np.asarray(tgt, np.float32)
    n, m = src.shape[0], tgt.shape[0]
    lhsT, rhs = _prep_aug(src, tgt)
    slots = build_slots(src, tgt)
    nslots = int(np.ceil(len(slots) / N_CORES))

    in_maps = []
    slot_of_core = []
    for core in range(N_CORES):
        lhs_a = np.zeros((K, nslots * P), np.float16)
        rhs_a = np.zeros((K, nslots * C), np.float16)
        rhs_a[:K_AUG] = rhs[:, m:m + 1]  # default: pad column
        csl = slots[core * nslots:(core + 1) * nslots]
        placed = []
        for si, pieces in enumerate(csl):
            ro = co = 0
            ranges = []
            pl = []
            for src_idx, cand_idx in pieces:
                lr, lc = len(src_idx), len(cand_idx)
                lhs_a[:K_AUG, si * P + ro:si * P + ro + lr] = lhsT[:, src_idx]
                rhs_a[:K_AUG, si * C + co:si * C + co + lc] = rhs[:, cand_idx]
                pl.append((src_idx, ro))
                ranges.append((ro, lr, co, lc))
                ro += lr
                co += lc
            used = co
            for g, (gro, glr, gco, glc) in enumerate(ranges):
                lhs_a[K_AUG + g, si * P + gro:si * P + gro + glr] = MASK_BIG
                rhs_a[K_AUG + g, si * C:si * C + used] = 1.0
                rhs_a[K_AUG + g, si * C + gco:si * C + gco + glc] = 0.0
            placed.append(pl)
        slot_of_core.append(placed)
        in_maps.append({"lhs": lhs_a, "rhs": rhs_a})

    nc = _get_nc(nslots)
    r = run_bass_kernel_spmd(nc, in_maps, list(range(N_CORES)), trace=trace)

    minq = np.full(n, np.inf, np.float32)
    for core in range(N_CORES):
        out = np.asarray(r.results[core]["out"])  # [P, nslots]
        for si, pl in enumerate(slot_of_core[core]):
            for src_idx, ro in pl:
                np.minimum.at(minq, src_idx, out[ro:ro + len(src_idx), si])
    return minq, r, nc


def _finish(minq, src):
    src = np.asarray(src, np.float32)
    s2 = (src.astype(np.float64) ** 2).sum(1).astype(np.float32)
    d2 = np.maximum(minq + s2, 0.0)
    clamped = np.where(d2 > 1.0, 0.0, d2)
    return np.float32(clamped.mean(dtype=np.float64))


def kernel(src, tgt, idx=None, **_ignored):
    minq, _, _ = _run_device(src, tgt)
    return np.asarray(_finish(minq, src))


def kernel_traced(src, tgt, idx=None):
    minq, r, nc = _run_device(src, tgt, trace=True)
    return np.asarray(_finish(minq, src)), r, nc


# revision 4
# speedup vs baseline: 1.1613x; 1.0323x over previous
"""Grid-accelerated KDTree-distance-loss kernel for Trainium2 (8 cores, SPMD).

Math: for each src point (16384 x 3), min over tgt (16384 x 3) of ||s-t||^2,
clamp (>1.0 -> 0), mean.

Algorithm (exact, clamp-aware):
  Host builds a uniform grid over tgt at cell sides h in {0.25, 0.5, 1.0}.
  For a src point in cell c (side h), every tgt outside the 3x3x3
  neighborhood N(c) is at distance >= h. A cheap host-side witness (distance
  to <=32 nearest-cell candidates) proves min <= h^2 for most src at
  h=0.25; the rest escalate. At the terminal level h=1.0, either
  min(candidates) <= 1.0 (exact) or the true min > 1.0 and the clamp sends
  it to 0 -- so three levels give the exact clamped loss.

  Src are Morton-walked and grown into blocks of <=128 points whose
  candidate-list union (bbox-ball filtered by the per-block witness
  radius) stays near the slot capacity; blocks are chunked to <=640
  columns and bin-packed into fixed 128-row x 640-col slots, with extra
  fp16 "mask rows" (+49152 at foreign columns) letting small blocks
  share a slot. The device computes, per slot, q[p, j] = -2 s_p . t_j
  + |t_j|^2 via an 11+6-row hi/lo-split fp16 matmul (PSUM fp32, ~1e-5
  abs accuracy) and reduces min_j q with one DVE tensor_tensor_scan over
  the two 320-halves of the PSUM tile (one read direct from PSUM, one
  staged to SBUF by the scalar engine). Host combines slot minima per
  src, adds |s|^2, clamps, means.
"""

import numpy as np

import concourse.bacc as bacc
import concourse.mybir as mybir
from concourse.tile import TileContext

N_CORES = 8
P = 128                # src points per block (partition dim)
C = 640                # candidate columns per slot
HALF = C // 2
G_MAX = 6              # max sub-blocks (pieces) per slot
K_AUG = 11             # hilo fp16 augmented rows
K = K_AUG + G_MAX      # + per-piece mask rows
LEVELS = (0.25, 0.5, 1.0)
WITNESS = 32
ORIGIN = -8.0
PAD_Q = 60000.0        # q value of the padding candidate column (fp16-safe)
MASK_BIG = 49152.0     # exact in fp16; added to q at foreign columns

_CACHE = {}


# ----------------------------------------------------------------- device ---

def build(nslots):
    """Bass module: nslots independent (<=128 src x C cand) min-reductions.
    Per slot: bank-aligned matmuls into one PSUM tile, Act stages the odd
    half to SBUF, one C/2-wide DVE min-scan folds both halves."""
    f16 = mybir.dt.float16
    f32 = mybir.dt.float32
    MIN = mybir.AluOpType.min

    nc = bacc.Bacc(None)
    lhs_d = nc.declare_dram_parameter("lhs", [K, nslots * P], f16, isOutput=False)
    rhs_d = nc.declare_dram_parameter("rhs", [K, nslots * C], f16, isOutput=False)
    out_d = nc.declare_dram_parameter("out", [P, nslots], f32, isOutput=True)

    with TileContext(nc) as tc:
        with (
            tc.tile_pool(name="const", bufs=1) as const_pool,
            tc.tile_pool(name="psum", bufs=4, space="PSUM") as psum_pool,
            tc.tile_pool(name="stage", bufs=4) as stage_pool,
        ):
            lhs = const_pool.tile([K, nslots * P], f16, tag="lhs")
            nc.sync.dma_start(lhs[:, :], lhs_d[:, :])
            rhs = const_pool.tile([K, nslots * C], f16, tag="rhs")
            # slot-aligned rhs chunks, small first so compute starts early
            chunks = []
            s0 = 0
            for sz in (1, 2, 4):
                if s0 + sz <= nslots:
                    chunks.append((s0, s0 + sz))
                    s0 += sz
            while s0 < nslots:
                sz = min(8, nslots - s0)
                chunks.append((s0, s0 + sz))
                s0 += sz
            for lo, hi in chunks:
                nc.sync.dma_start(rhs[:, lo * C:hi * C], rhs_d[:, lo * C:hi * C])
            so_all = const_pool.tile([P, nslots, HALF], f32, tag="so")

            half_s = nslots // 2
            for s in range(nslots):
                w = lhs[:, s * P:(s + 1) * P]
                p = psum_pool.tile([P, C], f32)
                # matmul writes must stay within a 512-fp32 PSUM bank
                for c0 in range(0, C, 512):
                    ce = min(C, c0 + 512)
                    nc.tensor.matmul(p[:, c0:ce], w,
                                     rhs[:, s * C + c0:s * C + ce],
                                     start=True, stop=True)
                c = stage_pool.tile([P, HALF], f32)
                nc.scalar.copy(c[:, :], p[:, HALF:C])
                nc.vector.tensor_tensor_scan(
                    out=so_all[:, s, :], data0=p[:, 0:HALF], data1=c[:, :],
                    initial=3.0e38, op0=MIN, op1=MIN,
                )
                if s == half_s - 1 and half_s > 0:
                    # first half of results leaves while the rest computes
                    nc.sync.dma_start(out_d[:, 0:half_s],
                                      so_all[:, 0:half_s, HALF - 1:HALF])
            if nslots - 1 > half_s:
                nc.sync.dma_start(out_d[:, half_s:nslots - 1],
                                  so_all[:, half_s:nslots - 1, HALF - 1:HALF])
            # tail DMA covers only the last slot: minimal post-scan chain
            nc.sync.dma_start(out_d[:, nslots - 1:nslots],
                              so_all[:, nslots - 1:nslots, HALF - 1:HALF])
    nc.compile()
    return nc


def _get_nc(nslots):
    key = ("nc", nslots)
    if key not in _CACHE:
        _CACHE[key] = build(nslots)
    return _CACHE[key]


# ------------------------------------------------------------ host indexing ---

def _morton(ci):
    def spread(x):
        x = x.astype(np.uint64)
        x = (x | (x << np.uint64(16))) & np.uint64(0x30000FF)
        x = (x | (x << np.uint64(8))) & np.uint64(0x300F00F)
        x = (x | (x << np.uint64(4))) & np.uint64(0x30C30C3)
        x = (x | (x << np.uint64(2))) & np.uint64(0x9249249)
        return x
    return (spread(ci[:, 0]) | (spread(ci[:, 1]) << np.uint64(1))
            | (spread(ci[:, 2]) << np.uint64(2)))


def _build_level(src_pts, tgt, h):
    nside = int(np.ceil(16.0 / h))
    ci_s = np.floor((np.clip(src_pts, -7.99, 7.99) - ORIGIN) / h).astype(np.int64)
    ci_t = np.floor((np.clip(tgt, -7.99, 7.99) - ORIGIN) / h).astype(np.int64)
    key_s = (ci_s[:, 0] * nside + ci_s[:, 1]) * nside + ci_s[:, 2]
    key_t = (ci_t[:, 0] * nside + ci_t[:, 1]) * nside + ci_t[:, 2]
    t_order = np.argsort(key_t, kind="stable")
    kt_sorted = key_t[t_order]
    trip = [(a, b, c) for a in (-1, 0, 1) for b in (-1, 0, 1) for c in (-1, 0, 1)]
    trip.sort(key=lambda t: abs(t[0]) + abs(t[1]) + abs(t[2]))
    offs = np.array([(a * nside + b) * nside + c for a, b, c in trip])
    return key_s, kt_sorted, t_order, offs, ci_s


def _cands_of_cell(u, kt_sorted, t_order, offs):
    segs = []
    for o in offs:
        lo = np.searchsorted(kt_sorted, u + o, side="left")
        hi = np.searchsorted(kt_sorted, u + o, side="right")
        if hi > lo:
            segs.append(t_order[lo:hi])
    return np.concatenate(segs) if segs else np.empty(0, np.int64)


def build_slots(src, tgt):
    """Returns slots: list of slot = list of pieces (src_idx<=P, cand_idx<=C).
    Pieces in one slot are from different blocks; mask rows keep them apart.

    Exactness: for each src s, its piece's candidate set contains every tgt
    within min(witness_dist(s), 1.0) of s (bbox-ball filter with radius
    r_blk = max over block members). So the computed min is the true min
    whenever the true min <= 1.0; otherwise both are > 1.0 -> clamp 0.
    """
    src64 = src.astype(np.float64)
    tgt64 = tgt.astype(np.float64)
    n = len(src64)
    remaining = np.arange(n)
    wit_d2 = np.full(n, np.inf)
    blocks = []  # (src_idx, cand_idx filtered)
    for li, h in enumerate(LEVELS):
        terminal = li == len(LEVELS) - 1
        if len(remaining) == 0:
            break
        key_s, kt_sorted, t_order, offs, ci_s = _build_level(src64[remaining], tgt64, h)
        uniq, inv = np.unique(key_s, return_inverse=True)
        cands = {u: _cands_of_cell(u, kt_sorted, t_order, offs) for u in uniq}
        guaranteed = np.zeros(len(remaining), bool)
        for i, u in enumerate(uniq):
            rows = np.where(inv == i)[0]
            cl = cands[u][:WITNESS]
            if len(cl) == 0:
                guaranteed[rows] = terminal
                continue
            d2 = ((src64[remaining[rows], None, :] - tgt64[None, cl, :]) ** 2
                  ).sum(-1).min(1)
            wit_d2[remaining[rows]] = np.minimum(wit_d2[remaining[rows]], d2)
            guaranteed[rows] = terminal or (d2 <= h * h)
        g_rows = np.where(guaranteed)[0]
        if len(g_rows):
            mort = _morton(ci_s[g_rows])
            g_sorted = g_rows[np.argsort(mort, kind="stable")]
            # adaptive blocks: grow along the Morton walk until either the
            # row capacity or the (unfiltered) union estimate is reached
            cap_unf = int(C * 1.45)
            member_groups = []
            mem = []
            cur = set()
            for m in g_sorted:
                add = cands[key_s[m]]
                new = cur | set(add.tolist())
                if mem and (len(mem) == P or len(new) > cap_unf):
                    member_groups.append(mem)
                    mem = [m]
                    cur = set(add.tolist())
                else:
                    mem.append(m)
                    cur = new
            if mem:
                member_groups.append(mem)
            for mem in member_groups:
                mem = np.asarray(mem)
                sidx = remaining[mem]
                cl = np.unique(np.concatenate([cands[key_s[m]] for m in mem]))
                if len(cl):
                    # bbox-ball filter: keep t with d(t, bbox)^2 <= r^2
                    pts = src64[sidx]
                    lo, hi = pts.min(0), pts.max(0)
                    r2 = np.minimum(wit_d2[sidx], 1.0).max() + 1e-9
                    tc = tgt64[cl]
                    dv = np.maximum(np.maximum(lo[None, :] - tc, tc - hi[None, :]), 0.0)
                    cl = cl[(dv ** 2).sum(1) <= r2]
                blocks.append((sidx, cl))
        remaining = remaining[~guaranteed]
    assert len(remaining) == 0

    # pieces: chunk block candidate lists to <= C columns
    pieces = []
    for bi, (src_idx, cl) in enumerate(blocks):
        if len(cl) == 0:
            cl = np.array([len(tgt)], np.int64)  # pad column only
        for c0 in range(0, len(cl), C):
            pieces.append((bi, src_idx, cl[c0:c0 + C]))
    # first-fit-decreasing bin packing: rows<=P, cols<=C, pieces<=G_MAX,
    # no two pieces of one block in the same slot
    pieces.sort(key=lambda t: -len(t[2]))
    slots = []  # each: [rows, cols, set(block_ids), [(src_idx, cand_idx)]]
    for bi, sidx, cl in pieces:
        for sl in slots:
            if (sl[0] + len(sidx) <= P and sl[1] + len(cl) <= C
                    and bi not in sl[2] and len(sl[3]) < G_MAX):
                sl[0] += len(sidx)
                sl[1] += len(cl)
                sl[2].add(bi)
                sl[3].append((sidx, cl))
                break
        else:
            slots.append([len(sidx), len(cl), {bi}, [(sidx, cl)]])
    return [sl[3] for sl in slots]


# ------------------------------------------------------------------- glue ---

def _prep_aug(src, tgt):
    """hilo fp16 augmentation. lhsT [K_AUG, N], rhs [K_AUG, M+1] (pad last)."""
    src = np.asarray(src, np.float32)
    tgt = np.asarray(tgt, np.float32)
    n, m = src.shape[0], tgt.shape[0]
    u = (-2.0 * tgt.astype(np.float64)).astype(np.float32)
    t2 = (tgt.astype(np.float64) ** 2).sum(1).astype(np.float32)
    hs = src.astype(np.float16)
    ls = (src - hs.astype(np.float32)).astype(np.float16)
    hu = u.astype(np.float16)
    lu = (u - hu.astype(np.float32)).astype(np.float16)
    t2h = t2.astype(np.float16)
    t2l = (t2 - t2h.astype(np.float32)).astype(np.float16)
    lhsT = np.empty((K_AUG, n), np.float16)
    lhsT[0:3] = hs.T
    lhsT[3:6] = ls.T
    lhsT[6:9] = hs.T
    lhsT[9] = np.float16(1.0)
    lhsT[10] = np.float16(1.0)
    rhs = np.zeros((K_AUG, m + 1), np.float16)
    rhs[0:3, :m] = hu.T
    rhs[3:6, :m] = hu.T
    rhs[6:9, :m] = lu.T
    rhs[9, :m] = t2h
    rhs[10, :m] = t2l
    rhs[9, m] = np.float16(PAD_Q)
    return lhsT, rhs


def _run_device(src, tgt, trace=False):
    from concourse.bass_utils import run_bass_kernel_spmd

    src = np.asarray(src, np.float32)
    tgt = np.asarray(tgt, np.float32)
    n, m = src.shape[0], tgt.shape[0]
    lhsT, rhs = _prep_aug(src, tgt)
    slots = build_slots(src, tgt)
    nslots = int(np.ceil(len(slots) / N_CORES))

    in_maps = []
    slot_of_core = []
    for core in range(N_CORES):
        lhs_a = np.zeros((K, nslots * P), np.float16)
        rhs_a = np.zeros((K, nslots * C), np.float16)
        rhs_a[:K_AUG] = rhs[:, m:m + 1]  # default: pad column
        csl = slots[core * nslots:(core + 1) * nslots]
        placed = []
        for si, pieces in enumerate(csl):
            ro = co = 0
            ranges = []
            pl = []
            for src_idx, cand_idx in pieces:
                lr, lc = len(src_idx), len(cand_idx)
                lhs_a[:K_AUG, si * P + ro:si * P + ro + lr] = lhsT[:, src_idx]
                rhs_a[:K_AUG, si * C + co:si * C + co + lc] = rhs[:, cand_idx]
                pl.append((src_idx, ro))
                ranges.append((ro, lr, co, lc))
                ro += lr
                co += lc
            used = co
            for g, (gro, glr, gco, glc) in enumerate(ranges):
                lhs_a[K_AUG + g, si * P + gro:si * P + gro + glr] = MASK_BIG
                rhs_a[K_AUG + g, si * C:si * C + used] = 1.0
                rhs_a[K_AUG + g, si * C + gco:si * C + gco + glc] = 0.0
            placed.append(pl)
        slot_of_core.append(placed)
        in_maps.append({"lhs": lhs_a, "rhs": rhs_a})

    nc = _get_nc(nslots)
    r = run_bass_kernel_spmd(nc, in_maps, list(range(N_CORES)), trace=trace)

    minq = np.full(n, np.inf, np.float32)
    for core in range(N_CORES):
        out = np.asarray(r.results[core]["out"])  # [P, nslots]
        for si, pl in enumerate(slot_of_core[core]):
            for src_idx, ro in pl:
                np.minimum.at(minq, src_idx, out[ro:ro + len(src_idx), si])
    return minq, r, nc


def _finish(minq, src):
    src = np.asarray(src, np.float32)
    s2 = (src.astype(np.float64) ** 2).sum(1).astype(np.float32)
    d2 = np.maximum(minq + s2, 0.0)
    clamped = np.where(d2 > 1.0, 0.0, d2)
    return np.float32(clamped.mean(dtype=np.float64))


def kernel(src, tgt, idx=None, **_ignored):
    minq, _, _ = _run_device(src, tgt)
    return np.asarray(_finish(minq, src))


def kernel_traced(src, tgt, idx=None):
    minq, r, nc = _run_device(src, tgt, trace=True)
    return np.asarray(_finish(minq, src)), r, nc


# revision 5
# speedup vs baseline: 1.2964x; 1.1163x over previous
"""Grid-accelerated KDTree-distance-loss kernel for Trainium2 (8 cores, SPMD).

Math: for each src point (16384 x 3), min over tgt (16384 x 3) of ||s-t||^2,
clamp (>1.0 -> 0), mean.

Algorithm (exact, clamp-aware):
  Host builds a uniform grid over tgt at cell sides h in {0.25, 0.5, 1.0}.
  For a src point in cell c (side h), every tgt outside the 3x3x3
  neighborhood N(c) is at distance >= h. A cheap host-side witness (distance
  to <=32 nearest-cell candidates) proves min <= h^2 for most src at
  h=0.25; the rest escalate. At the terminal level h=1.0, either
  min(candidates) <= 1.0 (exact) or the true min > 1.0 and the clamp sends
  it to 0 -- so three levels give the exact clamped loss.

  Src are Morton-walked and grown into blocks of <=128 points whose
  candidate-list union (bbox-ball filtered by the per-block witness
  radius) stays near the slot capacity; blocks are chunked to <=640
  columns and bin-packed into fixed 128-row x 640-col slots, with extra
  fp16 "mask rows" (+49152 at foreign columns) letting small blocks
  share a slot. The device computes, per slot, q[p, j] = -2 s_p . t_j
  + |t_j|^2 via an 11+6-row hi/lo-split fp16 matmul (PSUM fp32, ~1e-5
  abs accuracy) and reduces min_j q with one DVE tensor_tensor_scan over
  the two 320-halves of the PSUM tile (one read direct from PSUM, one
  staged to SBUF by the scalar engine). All operands arrive as a single
  combined input tensor whose first DMA chunk (lhs + slot-0 rhs) gates
  the first matmul. Host combines slot minima per src, adds |s|^2,
  clamps, means.
"""

import numpy as np

import concourse.bacc as bacc
import concourse.mybir as mybir
from concourse.tile import TileContext

N_CORES = 8
P = 128                # src points per block (partition dim)
C = 640                # candidate columns per slot
HALF = C // 2
G_MAX = 6              # max sub-blocks (pieces) per slot
K_AUG = 11             # hilo fp16 augmented rows
K = K_AUG + G_MAX      # + per-piece mask rows
LEVELS = (0.25, 0.5, 1.0)
WITNESS = 32
ORIGIN = -8.0
PAD_Q = 60000.0        # q value of the padding candidate column (fp16-safe)
MASK_BIG = 49152.0     # exact in fp16; added to q at foreign columns

_CACHE = {}


# ----------------------------------------------------------------- device ---

def build(nslots):
    """Bass module: nslots independent (<=128 src x 1024 cand) min-reductions.
    Per slot: two 512-col matmuls into one PSUM tile, Act stages the odd
    half to SBUF, one 512-wide DVE min-scan folds both halves."""
    f16 = mybir.dt.float16
    f32 = mybir.dt.float32
    MIN = mybir.AluOpType.min

    L0 = nslots * P
    nc = bacc.Bacc(None)
    inp_d = nc.declare_dram_parameter("inp", [K, L0 + nslots * C], f16,
                                      isOutput=False)
    out_d = nc.declare_dram_parameter("out", [P, nslots], f32, isOutput=True)

    with TileContext(nc) as tc:
        with (
            tc.tile_pool(name="const", bufs=1) as const_pool,
            tc.tile_pool(name="psum", bufs=4, space="PSUM") as psum_pool,
            tc.tile_pool(name="stage", bufs=4) as stage_pool,
        ):
            inp = const_pool.tile([K, L0 + nslots * C], f16, tag="inp")
            # one DMA covers all lhs + slot-0 rhs: a single latency chain
            # gates the first matmul; later rhs chunks stream in behind it
            nc.sync.dma_start(inp[:, 0:L0 + C], inp_d[:, 0:L0 + C])
            chunks = []
            s0 = 1
            for sz in (2, 4):
                if s0 + sz <= nslots:
                    chunks.append((s0, s0 + sz))
                    s0 += sz
            while s0 < nslots:
                sz = min(8, nslots - s0)
                chunks.append((s0, s0 + sz))
                s0 += sz
            for lo, hi in chunks:
                nc.sync.dma_start(inp[:, L0 + lo * C:L0 + hi * C],
                                  inp_d[:, L0 + lo * C:L0 + hi * C])
            so_all = const_pool.tile([P, nslots, HALF], f32, tag="so")

            half_s = nslots // 2
            for s in range(nslots):
                w = inp[:, s * P:(s + 1) * P]
                p = psum_pool.tile([P, C], f32)
                # matmul writes must stay within a 512-fp32 PSUM bank
                for c0 in range(0, C, 512):
                    ce = min(C, c0 + 512)
                    nc.tensor.matmul(p[:, c0:ce], w,
                                     inp[:, L0 + s * C + c0:L0 + s * C + ce],
                                     start=True, stop=True)
                c = stage_pool.tile([P, HALF], f32)
                nc.scalar.copy(c[:, :], p[:, HALF:C])
                nc.vector.tensor_tensor_scan(
                    out=so_all[:, s, :], data0=p[:, 0:HALF], data1=c[:, :],
                    initial=3.0e38, op0=MIN, op1=MIN,
                )
                if s == half_s - 1 and half_s > 0:
                    # first half of results leaves while the rest computes
                    nc.sync.dma_start(out_d[:, 0:half_s],
                                      so_all[:, 0:half_s, HALF - 1:HALF])
            if nslots - 1 > half_s:
                nc.sync.dma_start(out_d[:, half_s:nslots - 1],
                                  so_all[:, half_s:nslots - 1, HALF - 1:HALF])
            # tail DMA covers only the last slot: minimal post-scan chain
            nc.sync.dma_start(out_d[:, nslots - 1:nslots],
                              so_all[:, nslots - 1:nslots, HALF - 1:HALF])
    nc.compile()
    return nc


def _get_nc(nslots):
    key = ("nc", nslots)
    if key not in _CACHE:
        _CACHE[key] = build(nslots)
    return _CACHE[key]


# ------------------------------------------------------------ host indexing ---

def _morton(ci):
    def spread(x):
        x = x.astype(np.uint64)
        x = (x | (x << np.uint64(16))) & np.uint64(0x30000FF)
        x = (x | (x << np.uint64(8))) & np.uint64(0x300F00F)
        x = (x | (x << np.uint64(4))) & np.uint64(0x30C30C3)
        x = (x | (x << np.uint64(2))) & np.uint64(0x9249249)
        return x
    return (spread(ci[:, 0]) | (spread(ci[:, 1]) << np.uint64(1))
            | (spread(ci[:, 2]) << np.uint64(2)))


def _build_level(src_pts, tgt, h):
    nside = int(np.ceil(16.0 / h))
    ci_s = np.floor((np.clip(src_pts, -7.99, 7.99) - ORIGIN) / h).astype(np.int64)
    ci_t = np.floor((np.clip(tgt, -7.99, 7.99) - ORIGIN) / h).astype(np.int64)
    key_s = (ci_s[:, 0] * nside + ci_s[:, 1]) * nside + ci_s[:, 2]
    key_t = (ci_t[:, 0] * nside + ci_t[:, 1]) * nside + ci_t[:, 2]
    t_order = np.argsort(key_t, kind="stable")
    kt_sorted = key_t[t_order]
    trip = [(a, b, c) for a in (-1, 0, 1) for b in (-1, 0, 1) for c in (-1, 0, 1)]
    trip.sort(key=lambda t: abs(t[0]) + abs(t[1]) + abs(t[2]))
    offs = np.array([(a * nside + b) * nside + c for a, b, c in trip])
    return key_s, kt_sorted, t_order, offs, ci_s


def _cands_of_cell(u, kt_sorted, t_order, offs):
    segs = []
    for o in offs:
        lo = np.searchsorted(kt_sorted, u + o, side="left")
        hi = np.searchsorted(kt_sorted, u + o, side="right")
        if hi > lo:
            segs.append(t_order[lo:hi])
    return np.concatenate(segs) if segs else np.empty(0, np.int64)


def build_slots(src, tgt):
    """Returns slots: list of slot = list of pieces (src_idx<=P, cand_idx<=C).
    Pieces in one slot are from different blocks; mask rows keep them apart.

    Exactness: for each src s, its piece's candidate set contains every tgt
    within min(witness_dist(s), 1.0) of s (bbox-ball filter with radius
    r_blk = max over block members). So the computed min is the true min
    whenever the true min <= 1.0; otherwise both are > 1.0 -> clamp 0.
    """
    src64 = src.astype(np.float64)
    tgt64 = tgt.astype(np.float64)
    n = len(src64)
    remaining = np.arange(n)
    wit_d2 = np.full(n, np.inf)
    blocks = []  # (src_idx, cand_idx filtered)
    for li, h in enumerate(LEVELS):
        terminal = li == len(LEVELS) - 1
        if len(remaining) == 0:
            break
        key_s, kt_sorted, t_order, offs, ci_s = _build_level(src64[remaining], tgt64, h)
        uniq, inv = np.unique(key_s, return_inverse=True)
        cands = {u: _cands_of_cell(u, kt_sorted, t_order, offs) for u in uniq}
        guaranteed = np.zeros(len(remaining), bool)
        for i, u in enumerate(uniq):
            rows = np.where(inv == i)[0]
            cl = cands[u][:WITNESS]
            if len(cl) == 0:
                guaranteed[rows] = terminal
                continue
            d2 = ((src64[remaining[rows], None, :] - tgt64[None, cl, :]) ** 2
                  ).sum(-1).min(1)
            wit_d2[remaining[rows]] = np.minimum(wit_d2[remaining[rows]], d2)
            guaranteed[rows] = terminal or (d2 <= h * h)
        g_rows = np.where(guaranteed)[0]
        if len(g_rows):
            mort = _morton(ci_s[g_rows])
            g_sorted = g_rows[np.argsort(mort, kind="stable")]
            # adaptive blocks: grow along the Morton walk until either the
            # row capacity or the (unfiltered) union estimate is reached
            cap_unf = int(C * 1.45)
            member_groups = []
            mem = []
            cur = set()
            for m in g_sorted:
                add = cands[key_s[m]]
                new = cur | set(add.tolist())
                if mem and (len(mem) == P or len(new) > cap_unf):
                    member_groups.append(mem)
                    mem = [m]
                    cur = set(add.tolist())
                else:
                    mem.append(m)
                    cur = new
            if mem:
                member_groups.append(mem)
            for mem in member_groups:
                mem = np.asarray(mem)
                sidx = remaining[mem]
                cl = np.unique(np.concatenate([cands[key_s[m]] for m in mem]))
                if len(cl):
                    # bbox-ball filter: keep t with d(t, bbox)^2 <= r^2
                    pts = src64[sidx]
                    lo, hi = pts.min(0), pts.max(0)
                    r2 = np.minimum(wit_d2[sidx], 1.0).max() + 1e-9
                    tc = tgt64[cl]
                    dv = np.maximum(np.maximum(lo[None, :] - tc, tc - hi[None, :]), 0.0)
                    cl = cl[(dv ** 2).sum(1) <= r2]
                blocks.append((sidx, cl))
        remaining = remaining[~guaranteed]
    assert len(remaining) == 0

    # pieces: chunk block candidate lists to <= C columns
    pieces = []
    for bi, (src_idx, cl) in enumerate(blocks):
        if len(cl) == 0:
            cl = np.array([len(tgt)], np.int64)  # pad column only
        for c0 in range(0, len(cl), C):
            pieces.append((bi, src_idx, cl[c0:c0 + C]))
    # first-fit-decreasing bin packing: rows<=P, cols<=C, pieces<=G_MAX,
    # no two pieces of one block in the same slot
    pieces.sort(key=lambda t: -len(t[2]))
    slots = []  # each: [rows, cols, set(block_ids), [(src_idx, cand_idx)]]
    for bi, sidx, cl in pieces:
        for sl in slots:
            if (sl[0] + len(sidx) <= P and sl[1] + len(cl) <= C
                    and bi not in sl[2] and len(sl[3]) < G_MAX):
                sl[0] += len(sidx)
                sl[1] += len(cl)
                sl[2].add(bi)
                sl[3].append((sidx, cl))
                break
        else:
            slots.append([len(sidx), len(cl), {bi}, [(sidx, cl)]])
    return [sl[3] for sl in slots]


# ------------------------------------------------------------------- glue ---

def _prep_aug(src, tgt):
    """hilo fp16 augmentation. lhsT [K_AUG, N], rhs [K_AUG, M+1] (pad last)."""
    src = np.asarray(src, np.float32)
    tgt = np.asarray(tgt, np.float32)
    n, m = src.shape[0], tgt.shape[0]
    u = (-2.0 * tgt.astype(np.float64)).astype(np.float32)
    t2 = (tgt.astype(np.float64) ** 2).sum(1).astype(np.float32)
    hs = src.astype(np.float16)
    ls = (src - hs.astype(np.float32)).astype(np.float16)
    hu = u.astype(np.float16)
    lu = (u - hu.astype(np.float32)).astype(np.float16)
    t2h = t2.astype(np.float16)
    t2l = (t2 - t2h.astype(np.float32)).astype(np.float16)
    lhsT = np.empty((K_AUG, n), np.float16)
    lhsT[0:3] = hs.T
    lhsT[3:6] = ls.T
    lhsT[6:9] = hs.T
    lhsT[9] = np.float16(1.0)
    lhsT[10] = np.float16(1.0)
    rhs = np.zeros((K_AUG, m + 1), np.float16)
    rhs[0:3, :m] = hu.T
    rhs[3:6, :m] = hu.T
    rhs[6:9, :m] = lu.T
    rhs[9, :m] = t2h
    rhs[10, :m] = t2l
    rhs[9, m] = np.float16(PAD_Q)
    return lhsT, rhs


def _run_device(src, tgt, trace=False):
    from concourse.bass_utils import run_bass_kernel_spmd

    src = np.asarray(src, np.float32)
    tgt = np.asarray(tgt, np.float32)
    n, m = src.shape[0], tgt.shape[0]
    lhsT, rhs = _prep_aug(src, tgt)
    slots = build_slots(src, tgt)
    nslots = int(np.ceil(len(slots) / N_CORES))

    in_maps = []
    slot_of_core = []
    for core in range(N_CORES):
        lhs_a = np.zeros((K, nslots * P), np.float16)
        rhs_a = np.zeros((K, nslots * C), np.float16)
        rhs_a[:K_AUG] = rhs[:, m:m + 1]  # default: pad column
        csl = slots[core * nslots:(core + 1) * nslots]
        placed = []
        for si, pieces in enumerate(csl):
            ro = co = 0
            ranges = []
            pl = []
            for src_idx, cand_idx in pieces:
                lr, lc = len(src_idx), len(cand_idx)
                lhs_a[:K_AUG, si * P + ro:si * P + ro + lr] = lhsT[:, src_idx]
                rhs_a[:K_AUG, si * C + co:si * C + co + lc] = rhs[:, cand_idx]
                pl.append((src_idx, ro))
                ranges.append((ro, lr, co, lc))
                ro += lr
                co += lc
            used = co
            for g, (gro, glr, gco, glc) in enumerate(ranges):
                lhs_a[K_AUG + g, si * P + gro:si * P + gro + glr] = MASK_BIG
                rhs_a[K_AUG + g, si * C:si * C + used] = 1.0
                rhs_a[K_AUG + g, si * C + gco:si * C + gco + glc] = 0.0
            placed.append(pl)
        slot_of_core.append(placed)
        in_maps.append({"inp": np.concatenate([lhs_a, rhs_a], axis=1)})

    nc = _get_nc(nslots)
    r = run_bass_kernel_spmd(nc, in_maps, list(range(N_CORES)), trace=trace)

    minq = np.full(n, np.inf, np.float32)
    for core in range(N_CORES):
        out = np.asarray(r.results[core]["out"])  # [P, nslots]
        for si, pl in enumerate(slot_of_core[core]):
            for src_idx, ro in pl:
                np.minimum.at(minq, src_idx, out[ro:ro + len(src_idx), si])
    return minq, r, nc


def _finish(minq, src):
    src = np.asarray(src, np.float32)
    s2 = (src.astype(np.float64) ** 2).sum(1).astype(np.float32)
    d2 = np.maximum(minq + s2, 0.0)
    clamped = np.where(d2 > 1.0, 0.0, d2)
    return np.float32(clamped.mean(dtype=np.float64))


def kernel(src, tgt, idx=None, **_ignored):
    minq, _, _ = _run_device(src, tgt)
    return np.asarray(_finish(minq, src))


def kernel_traced(src, tgt, idx=None):
    minq, r, nc = _run_device(src, tgt, trace=True)
    return np.asarray(_finish(minq, src)), r, nc


# revision 6
# speedup vs baseline: 1.3227x; 1.0203x over previous
"""Grid-accelerated KDTree-distance-loss kernel for Trainium2 (8 cores, SPMD).

Math: for each src point (16384 x 3), min over tgt (16384 x 3) of ||s-t||^2,
clamp (>1.0 -> 0), mean.

Algorithm (exact, clamp-aware):
  Host builds a uniform grid over tgt at cell sides h in {0.25, 0.5, 1.0}.
  For a src point in cell c (side h), every tgt outside the 3x3x3
  neighborhood N(c) is at distance >= h. A cheap host-side witness (distance
  to <=32 nearest-cell candidates) proves min <= h^2 for most src at
  h=0.25; the rest escalate. At the terminal level h=1.0, either
  min(candidates) <= 1.0 (exact) or the true min > 1.0 and the clamp sends
  it to 0 -- so three levels give the exact clamped loss.

  Src are Morton-walked and grown into blocks of <=128 points whose
  candidate-list union (bbox-ball filtered by the per-block witness
  radius) stays near the slot capacity; blocks are chunked to <=640
  columns and bin-packed into fixed 128-row x 640-col slots, with extra
  fp16 "mask rows" (+49152 at foreign columns) letting up to 8 blocks
  share a slot. The device computes, per slot, q[p, j] = -2 s_p . t_j
  + |t_j|^2 via an 11+8-row hi/lo-split fp16 matmul (PSUM fp32, ~1e-5
  abs accuracy) and reduces min_j q with one DVE tensor_tensor_scan over
  the two 320-halves of the PSUM tile (one read direct from PSUM, one
  staged to SBUF by the scalar engine). All operands arrive as a single
  combined input tensor whose first DMA chunk (lhs + slot-0 rhs) gates
  the first matmul. Host combines slot minima per src, adds |s|^2,
  clamps, means.
"""

import numpy as np

import concourse.bacc as bacc
import concourse.mybir as mybir
from concourse.tile import TileContext

N_CORES = 8
P = 128                # src points per block (partition dim)
C = 640                # candidate columns per slot
HALF = C // 2
G_MAX = 8              # max sub-blocks (pieces) per slot
K_AUG = 11             # hilo fp16 augmented rows
K = K_AUG + G_MAX      # + per-piece mask rows
LEVELS = (0.25, 0.5, 1.0)
WITNESS = 64
ORIGIN = -8.0
PAD_Q = 60000.0        # q value of the padding candidate column (fp16-safe)
MASK_BIG = 49152.0     # exact in fp16; added to q at foreign columns

_CACHE = {}


# ----------------------------------------------------------------- device ---

def build(nslots):
    """Bass module: nslots independent (<=128 src x 1024 cand) min-reductions.
    Per slot: two 512-col matmuls into one PSUM tile, Act stages the odd
    half to SBUF, one 512-wide DVE min-scan folds both halves."""
    f16 = mybir.dt.float16
    f32 = mybir.dt.float32
    MIN = mybir.AluOpType.min

    L0 = nslots * P
    nc = bacc.Bacc(None)
    inp_d = nc.declare_dram_parameter("inp", [K, L0 + nslots * C], f16,
                                      isOutput=False)
    out_d = nc.declare_dram_parameter("out", [P, nslots], f32, isOutput=True)

    with TileContext(nc) as tc:
        with (
            tc.tile_pool(name="const", bufs=1) as const_pool,
            tc.tile_pool(name="psum", bufs=4, space="PSUM") as psum_pool,
            tc.tile_pool(name="stage", bufs=4) as stage_pool,
        ):
            inp = const_pool.tile([K, L0 + nslots * C], f16, tag="inp")
            # one DMA covers all lhs + slot-0 rhs: a single latency chain
            # gates the first matmul; later rhs chunks stream in behind it
            nc.sync.dma_start(inp[:, 0:L0 + C], inp_d[:, 0:L0 + C])
            chunks = []
            s0 = 1
            for sz in (2, 4):
                if s0 + sz <= nslots:
                    chunks.append((s0, s0 + sz))
                    s0 += sz
            while s0 < nslots:
                sz = min(8, nslots - s0)
                chunks.append((s0, s0 + sz))
                s0 += sz
            for lo, hi in chunks:
                nc.sync.dma_start(inp[:, L0 + lo * C:L0 + hi * C],
                                  inp_d[:, L0 + lo * C:L0 + hi * C])
            so_all = const_pool.tile([P, nslots, HALF], f32, tag="so")

            half_s = nslots // 2
            for s in range(nslots):
                w = inp[:, s * P:(s + 1) * P]
                p = psum_pool.tile([P, C], f32)
                # matmul writes must stay within a 512-fp32 PSUM bank
                for c0 in range(0, C, 512):
                    ce = min(C, c0 + 512)
                    nc.tensor.matmul(p[:, c0:ce], w,
                                     inp[:, L0 + s * C + c0:L0 + s * C + ce],
                                     start=True, stop=True)
                c = stage_pool.tile([P, HALF], f32)
                nc.scalar.copy(c[:, :], p[:, HALF:C])
                nc.vector.tensor_tensor_scan(
                    out=so_all[:, s, :], data0=p[:, 0:HALF], data1=c[:, :],
                    initial=3.0e38, op0=MIN, op1=MIN,
                )
                if s == half_s - 1 and half_s > 0:
                    # first half of results leaves while the rest computes
                    nc.sync.dma_start(out_d[:, 0:half_s],
                                      so_all[:, 0:half_s, HALF - 1:HALF])
            if nslots - 1 > half_s:
                nc.sync.dma_start(out_d[:, half_s:nslots - 1],
                                  so_all[:, half_s:nslots - 1, HALF - 1:HALF])
            # tail DMA covers only the last slot: minimal post-scan chain
            nc.sync.dma_start(out_d[:, nslots - 1:nslots],
                              so_all[:, nslots - 1:nslots, HALF - 1:HALF])
    nc.compile()
    return nc


def _get_nc(nslots):
    key = ("nc", nslots)
    if key not in _CACHE:
        _CACHE[key] = build(nslots)
    return _CACHE[key]


# ------------------------------------------------------------ host indexing ---

def _morton(ci):
    def spread(x):
        x = x.astype(np.uint64)
        x = (x | (x << np.uint64(16))) & np.uint64(0x30000FF)
        x = (x | (x << np.uint64(8))) & np.uint64(0x300F00F)
        x = (x | (x << np.uint64(4))) & np.uint64(0x30C30C3)
        x = (x | (x << np.uint64(2))) & np.uint64(0x9249249)
        return x
    return (spread(ci[:, 0]) | (spread(ci[:, 1]) << np.uint64(1))
            | (spread(ci[:, 2]) << np.uint64(2)))


def _build_level(src_pts, tgt, h):
    nside = int(np.ceil(16.0 / h))
    ci_s = np.floor((np.clip(src_pts, -7.99, 7.99) - ORIGIN) / h).astype(np.int64)
    ci_t = np.floor((np.clip(tgt, -7.99, 7.99) - ORIGIN) / h).astype(np.int64)
    key_s = (ci_s[:, 0] * nside + ci_s[:, 1]) * nside + ci_s[:, 2]
    key_t = (ci_t[:, 0] * nside + ci_t[:, 1]) * nside + ci_t[:, 2]
    t_order = np.argsort(key_t, kind="stable")
    kt_sorted = key_t[t_order]
    trip = [(a, b, c) for a in (-1, 0, 1) for b in (-1, 0, 1) for c in (-1, 0, 1)]
    trip.sort(key=lambda t: abs(t[0]) + abs(t[1]) + abs(t[2]))
    offs = np.array([(a * nside + b) * nside + c for a, b, c in trip])
    return key_s, kt_sorted, t_order, offs, ci_s


def _cands_of_cell(u, kt_sorted, t_order, offs):
    segs = []
    for o in offs:
        lo = np.searchsorted(kt_sorted, u + o, side="left")
        hi = np.searchsorted(kt_sorted, u + o, side="right")
        if hi > lo:
            segs.append(t_order[lo:hi])
    return np.concatenate(segs) if segs else np.empty(0, np.int64)


def build_slots(src, tgt):
    """Returns slots: list of slot = list of pieces (src_idx<=P, cand_idx<=C).
    Pieces in one slot are from different blocks; mask rows keep them apart.

    Exactness: for each src s, its piece's candidate set contains every tgt
    within min(witness_dist(s), 1.0) of s (bbox-ball filter with radius
    r_blk = max over block members). So the computed min is the true min
    whenever the true min <= 1.0; otherwise both are > 1.0 -> clamp 0.
    """
    src64 = src.astype(np.float64)
    tgt64 = tgt.astype(np.float64)
    n = len(src64)
    remaining = np.arange(n)
    wit_d2 = np.full(n, np.inf)
    blocks = []  # (src_idx, cand_idx filtered)
    for li, h in enumerate(LEVELS):
        terminal = li == len(LEVELS) - 1
        if len(remaining) == 0:
            break
        key_s, kt_sorted, t_order, offs, ci_s = _build_level(src64[remaining], tgt64, h)
        uniq, inv = np.unique(key_s, return_inverse=True)
        cands = {u: _cands_of_cell(u, kt_sorted, t_order, offs) for u in uniq}
        guaranteed = np.zeros(len(remaining), bool)
        for i, u in enumerate(uniq):
            rows = np.where(inv == i)[0]
            cl = cands[u][:WITNESS]
            if len(cl) == 0:
                guaranteed[rows] = terminal
                continue
            d2 = ((src64[remaining[rows], None, :] - tgt64[None, cl, :]) ** 2
                  ).sum(-1).min(1)
            wit_d2[remaining[rows]] = np.minimum(wit_d2[remaining[rows]], d2)
            guaranteed[rows] = terminal or (d2 <= h * h)
        g_rows = np.where(guaranteed)[0]
        if len(g_rows):
            mort = _morton(ci_s[g_rows])
            g_sorted = g_rows[np.argsort(mort, kind="stable")]
            # adaptive blocks: grow along the Morton walk until either the
            # row capacity or the (unfiltered) union estimate is reached
            cap_unf = int(C * 0.78)
            member_groups = []
            mem = []
            cur = set()
            for m in g_sorted:
                add = cands[key_s[m]]
                new = cur | set(add.tolist())
                if mem and (len(mem) == P or len(new) > cap_unf):
                    member_groups.append(mem)
                    mem = [m]
                    cur = set(add.tolist())
                else:
                    mem.append(m)
                    cur = new
            if mem:
                member_groups.append(mem)
            for mem in member_groups:
                mem = np.asarray(mem)
                sidx = remaining[mem]
                cl = np.unique(np.concatenate([cands[key_s[m]] for m in mem]))
                if len(cl):
                    # bbox-ball filter: keep t with d(t, bbox)^2 <= r^2
                    pts = src64[sidx]
                    lo, hi = pts.min(0), pts.max(0)
                    r2 = np.minimum(wit_d2[sidx], 1.0).max() + 1e-9
                    tc = tgt64[cl]
                    dv = np.maximum(np.maximum(lo[None, :] - tc, tc - hi[None, :]), 0.0)
                    cl = cl[(dv ** 2).sum(1) <= r2]
                blocks.append((sidx, cl))
        remaining = remaining[~guaranteed]
    assert len(remaining) == 0

    # pieces: chunk block candidate lists to <= C columns
    pieces = []
    for bi, (src_idx, cl) in enumerate(blocks):
        if len(cl) == 0:
            cl = np.array([len(tgt)], np.int64)  # pad column only
        for c0 in range(0, len(cl), C):
            pieces.append((bi, src_idx, cl[c0:c0 + C]))
    # first-fit-decreasing bin packing: rows<=P, cols<=C, pieces<=G_MAX,
    # no two pieces of one block in the same slot
    pieces.sort(key=lambda t: -len(t[2]))
    slots = []  # each: [rows, cols, set(block_ids), [(src_idx, cand_idx)]]
    for bi, sidx, cl in pieces:
        for sl in slots:
            if (sl[0] + len(sidx) <= P and sl[1] + len(cl) <= C
                    and bi not in sl[2] and len(sl[3]) < G_MAX):
                sl[0] += len(sidx)
                sl[1] += len(cl)
                sl[2].add(bi)
                sl[3].append((sidx, cl))
                break
        else:
            slots.append([len(sidx), len(cl), {bi}, [(sidx, cl)]])
    return [sl[3] for sl in slots]


# ------------------------------------------------------------------- glue ---

def _prep_aug(src, tgt):
    """hilo fp16 augmentation. lhsT [K_AUG, N], rhs [K_AUG, M+1] (pad last)."""
    src = np.asarray(src, np.float32)
    tgt = np.asarray(tgt, np.float32)
    n, m = src.shape[0], tgt.shape[0]
    u = (-2.0 * tgt.astype(np.float64)).astype(np.float32)
    t2 = (tgt.astype(np.float64) ** 2).sum(1).astype(np.float32)
    hs = src.astype(np.float16)
    ls = (src - hs.astype(np.float32)).astype(np.float16)
    hu = u.astype(np.float16)
    lu = (u - hu.astype(np.float32)).astype(np.float16)
    t2h = t2.astype(np.float16)
    t2l = (t2 - t2h.astype(np.float32)).astype(np.float16)
    lhsT = np.empty((K_AUG, n), np.float16)
    lhsT[0:3] = hs.T
    lhsT[3:6] = ls.T
    lhsT[6:9] = hs.T
    lhsT[9] = np.float16(1.0)
    lhsT[10] = np.float16(1.0)
    rhs = np.zeros((K_AUG, m + 1), np.float16)
    rhs[0:3, :m] = hu.T
    rhs[3:6, :m] = hu.T
    rhs[6:9, :m] = lu.T
    rhs[9, :m] = t2h
    rhs[10, :m] = t2l
    rhs[9, m] = np.float16(PAD_Q)
    return lhsT, rhs


def _run_device(src, tgt, trace=False):
    from concourse.bass_utils import run_bass_kernel_spmd

    src = np.asarray(src, np.float32)
    tgt = np.asarray(tgt, np.float32)
    n, m = src.shape[0], tgt.shape[0]
    lhsT, rhs = _prep_aug(src, tgt)
    slots = build_slots(src, tgt)
    nslots = int(np.ceil(len(slots) / N_CORES))

    in_maps = []
    slot_of_core = []
    for core in range(N_CORES):
        lhs_a = np.zeros((K, nslots * P), np.float16)
        rhs_a = np.zeros((K, nslots * C), np.float16)
        rhs_a[:K_AUG] = rhs[:, m:m + 1]  # default: pad column
        csl = slots[core * nslots:(core + 1) * nslots]
        placed = []
        for si, pieces in enumerate(csl):
            ro = co = 0
            ranges = []
            pl = []
            for src_idx, cand_idx in pieces:
                lr, lc = len(src_idx), len(cand_idx)
                lhs_a[:K_AUG, si * P + ro:si * P + ro + lr] = lhsT[:, src_idx]
                rhs_a[:K_AUG, si * C + co:si * C + co + lc] = rhs[:, cand_idx]
                pl.append((src_idx, ro))
                ranges.append((ro, lr, co, lc))
                ro += lr
                co += lc
            used = co
            for g, (gro, glr, gco, glc) in enumerate(ranges):
                lhs_a[K_AUG + g, si * P + gro:si * P + gro + glr] = MASK_BIG
                rhs_a[K_AUG + g, si * C:si * C + used] = 1.0
                rhs_a[K_AUG + g, si * C + gco:si * C + gco + glc] = 0.0
            placed.append(pl)
        slot_of_core.append(placed)
        in_maps.append({"inp": np.concatenate([lhs_a, rhs_a], axis=1)})

    nc = _get_nc(nslots)
    r = run_bass_kernel_spmd(nc, in_maps, list(range(N_CORES)), trace=trace)

    minq = np.full(n, np.inf, np.float32)
    for core in range(N_CORES):
        out = np.asarray(r.results[core]["out"])  # [P, nslots]
        for si, pl in enumerate(slot_of_core[core]):
            for src_idx, ro in pl:
                np.minimum.at(minq, src_idx, out[ro:ro + len(src_idx), si])
    return minq, r, nc


def _finish(minq, src):
    src = np.asarray(src, np.float32)
    s2 = (src.astype(np.float64) ** 2).sum(1).astype(np.float32)
    d2 = np.maximum(minq + s2, 0.0)
    clamped = np.where(d2 > 1.0, 0.0, d2)
    return np.float32(clamped.mean(dtype=np.float64))


def kernel(src, tgt, idx=None, **_ignored):
    minq, _, _ = _run_device(src, tgt)
    return np.asarray(_finish(minq, src))


def kernel_traced(src, tgt, idx=None):
    minq, r, nc = _run_device(src, tgt, trace=True)
    return np.asarray(_finish(minq, src)), r, nc
